# revision 1
# baseline (speedup 1.0000x reference)
"""EntityEncoder (gnn_message_passing) Trainium2 kernel — 8-core SPMD.

Strategy: edges are pre-partitioned on the host into 8 contiguous,
entity-aligned, edge-balanced shards (entity_indices is sorted, so each
entity's edges land wholly on one core — no cross-core collectives).
Within a core, segments are LPT-packed into 10 blocks of <=128 segments /
<=1280 edges; segment softmax + weighted segment-sums run as one-hot
matmuls on the tensor engine; count-table aggregation goes through an
A-matrix (segment x count) contracted against count_table; both output
projections run as bf16 matmuls against host-transposed weights.
"""
import sys
import numpy as np
import ml_dtypes

for _p in ("/root/.axon_site", "/root/.axon_site/_ro/trn_rl_repo",
           "/root/.axon_site/_ro/pypackages"):
    if _p not in sys.path:
        sys.path.append(_p)

import bass_rust
import concourse.bass as bass
import concourse.mybir as mybir
import concourse.tile as tile
from concourse.vector_clock import ScopedClock
from contextlib import ExitStack

BF16 = ml_dtypes.bfloat16
dt = mybir.dt
Alu = mybir.AluOpType
Act = mybir.ActivationFunctionType

# problem shape (hardcoded per contest contract)
N_CORES = 8
N = 100_000
P = 64
E = 10_000
D = 768
C = 1000
CPAD = 1024
OUT = 5120
# per-core packing
NBLK = 10
SPB = 128                # segs per block
CH = 10                  # chunks (of 128 edges) per block
EPB = CH * 128           # edges per block = 1280
NL = NBLK * EPB          # 12800 edge slots per core
E_PAD = NBLK * SPB       # 1280 seg slots per core
OH = OUT // 5            # 1024-wide output slab
PAD_SEG = 999.0


class _TileContextSplitDrain(tile.TileContext):
    """This container's walrus accepts only ONE sync wait per instruction
    ("Too many sync wait commands" in setupSyncWait). Split every extra wait
    onto a standalone same-engine NoOp placed immediately before the
    instruction — identical semantics, one wait per instruction."""

    def _lower_ordered_insts(self, ordered):
        for insts in ordered.values():
            if not any(
                i.sync_info is not None and len(i.sync_info.on_wait) > 1
                for i in insts
            ):
                continue
            new = []
            for inst in insts:
                si = inst.sync_info
                if si is not None and len(si.on_wait) > 1:
                    waits = list(si.on_wait)
                    for w in waits[:-1]:
                        nop = bass_rust.InstNoOp(
                            name=self.nc.get_next_instruction_name(),
                            ins=[], outs=[])
                        nop.engine = inst.engine
                        nop.sync_info = bass_rust.SyncInfo(
                            on_wait=[w], on_update=[])
                        new.append(nop)
                    si.on_wait = waits[-1:]
                new.append(inst)
            insts[:] = new
        return super()._lower_ordered_insts(ordered)

    def _drain_and_barrier(self, tick_clock, wait_clock):
        nc = self.nc
        drain_inst = nc.sync.drain()
        wait_clock.add_sem_waits(
            drain_inst.ins, ScopedClock({None: tick_clock.global_clock})
        )
        si = drain_inst.ins.sync_info
        if si is not None and len(si.on_wait) > 1:
            waits = list(si.on_wait)
            si.on_wait = waits[:1]
            for w in waits[1:]:
                n = nc.sync.nop()
                n.ins.sync_info = bass_rust.SyncInfo(on_wait=[w], on_update=[])
        nc.all_engine_barrier()
        assert self.sems is not None
        popped = nc._tile_sem_poison_stack.pop()
        assert popped is self._sem_poison
        nc.clear_and_free_semaphores(list(self.sems.allocated().values()))
        nc.all_engine_barrier()


# --------------------------------------------------------------------------
# host-side sharding / packing
# --------------------------------------------------------------------------

def _shard_and_pack(entity_indices):
    Nn = entity_indices.shape[0]
    starts = np.searchsorted(entity_indices, np.arange(E + 1))
    ideal = (np.arange(1, N_CORES) * Nn) // N_CORES
    ent_bnd = [0]
    for t in ideal:
        s = int(np.searchsorted(starts, t))
        if s > 0 and abs(int(starts[s - 1]) - int(t)) < abs(int(starts[s]) - int(t)):
            s -= 1
        ent_bnd.append(s)
    ent_bnd.append(E)

    cores = []
    for c in range(N_CORES):
        e_lo, e_hi = ent_bnd[c], ent_bnd[c + 1]
        segs = np.arange(e_lo, e_hi)
        sizes = (starts[e_lo + 1 : e_hi + 1] - starts[e_lo:e_hi]).astype(np.int64)
        n_edges = int(sizes.sum())
        assert e_hi - e_lo <= E_PAD and n_edges <= NL
        order = np.argsort(-sizes, kind="stable")
        blk_edges = [0] * NBLK
        blk_nseg = [0] * NBLK
        blk_segs = [[] for _ in range(NBLK)]
        for idx in order:
            sz = int(sizes[idx])
            best = -1
            for b in sorted(range(NBLK), key=lambda b: blk_edges[b]):
                if blk_nseg[b] < SPB and blk_edges[b] + sz <= EPB:
                    best = b
                    break
            assert best >= 0, "block packing overflow"
            blk_segs[best].append(int(segs[idx]))
            blk_edges[best] += sz
            blk_nseg[best] += 1
        perm = np.full(NL, -1, dtype=np.int64)
        seg_local = np.full(NL, PAD_SEG, dtype=np.float32)
        row2seg = np.full(E_PAD, -1, dtype=np.int64)
        inv_cnt = np.zeros(E_PAD, dtype=np.float32)
        for b in range(NBLK):
            pos = b * EPB
            for j, s in enumerate(blk_segs[b]):
                row = b * SPB + j
                row2seg[row] = s
                n = int(starts[s + 1] - starts[s])
                if n > 0:
                    inv_cnt[row] = 1.0 / n
                perm[pos : pos + n] = np.arange(starts[s], starts[s + 1])
                seg_local[pos : pos + n] = float(j)
                pos += n
        cores.append(dict(perm=perm, seg_local=seg_local, row2seg=row2seg,
                          inv_cnt=inv_cnt))
    return cores


# --------------------------------------------------------------------------
# device kernel
# --------------------------------------------------------------------------

def _build_nc():
    nc = bass.Bass("TRN2", target_bir_lowering=False, debug=False,
                   num_devices=N_CORES)

    f32, bf, f16, i32 = dt.float32, dt.bfloat16, dt.float16, dt.int32
    din = lambda n, s, d=f32: nc.dram_tensor(n, s, d, kind="ExternalInput")
    ent_d = din("ent", [NL, D])
    nbr_d = din("nbr", [NL, D])
    rel_d = din("rel", [NL, D])
    segl_d = din("segl", [NL])
    cnt_d = din("cntf", [NL])
    pr_d = din("prf", [NL])
    icnt_d = din("inv_cnt", [E_PAD])
    cscb_d = din("cscb", [128, CPAD], bf)
    pscb_d = din("pscb", [128, P], bf)
    wse_d = din("wse", [128, D], bf)
    wsn_d = din("wsn", [128, D], bf)
    wsr_d = din("wsr", [128, D], bf)
    ctp_d = din("ctp", [CPAD, D])
    wtr_d = din("wtr", [2 * D, OUT])
    wte_d = din("wte", [D, OUT])
    brel_d = din("brel", [OUT])
    bent_d = din("bent", [OUT])
    orel_d = nc.dram_tensor("orel", [E_PAD, OUT], f32, kind="ExternalOutput")
    oent_d = nc.dram_tensor("oent", [E_PAD, OUT], f32, kind="ExternalOutput")

    with _TileContextSplitDrain(nc) as tc, ExitStack() as es:
        const = es.enter_context(tc.tile_pool(name="const", bufs=1))
        accp = es.enter_context(tc.tile_pool(name="accp", bufs=1))

        # ---- constants ----
        iota_cnt = const.tile([128, CPAD], f16)
        iota_seg = const.tile([128, 128], bf)
        ident = const.tile([128, 128], bf)
        with tc.tile_pool(name="setup", bufs=1) as setup:
            iota_i = setup.tile([128, CPAD], i32)
            nc.gpsimd.iota(iota_i[:], pattern=[[1, CPAD]], base=0,
                           channel_multiplier=0)
            nc.vector.tensor_copy(iota_cnt[:], iota_i[:])
            nc.vector.tensor_copy(iota_seg[:], iota_i[:, 0:128])
            iota_ci = setup.tile([128, 1], i32)
            nc.gpsimd.iota(iota_ci[:], pattern=[[0, 1]], base=0,
                           channel_multiplier=1)
            iota_col = setup.tile([128, 1], f32)
            nc.vector.tensor_copy(iota_col[:], iota_ci[:])
            nc.vector.tensor_scalar(out=ident[:], in0=iota_seg[:],
                                    scalar1=iota_col[:],
                                    scalar2=None, op0=Alu.is_equal)
        ones_r = const.tile([1, 128], bf)
        nc.vector.memset(ones_r[:], 1.0)

        wse = const.tile([128, D], bf)
        nc.sync.dma_start(wse[:], wse_d.ap())
        wsn = const.tile([128, D], bf)
        nc.sync.dma_start(wsn[:], wsn_d.ap())
        wsr = const.tile([128, D], bf)
        nc.sync.dma_start(wsr[:], wsr_d.ap())
        cscb = const.tile([128, CPAD], bf)
        nc.sync.dma_start(cscb[:], cscb_d.ap())
        pscb = const.tile([128, P], bf)
        nc.sync.dma_start(pscb[:], pscb_d.ap())
        ctsb = const.tile([128, 8 * D], bf)
        nc.gpsimd.dma_start(
            ctsb[:], ctp_d.ap().rearrange("(i p) d -> p i d", p=128)
        )
        icnt_sb = const.tile([128, NBLK], f32)
        nc.sync.dma_start(
            icnt_sb[:], icnt_d.ap().rearrange("(b p) -> p b", p=128)
        )
        invd_sb = accp.tile([128, NBLK], f32)

        # resident transposed aggregates, one tile per (feat-chunk, block) so
        # projection reads depend only on their own block's writes
        relcatT = [[accp.tile([128, 128], bf, name=f"relcatT{t}_{b}",
                              tag=f"relcatT{t}_{b}") for b in range(NBLK)]
                   for t in range(12)]
        entT = [[accp.tile([128, 128], bf, name=f"entT{t}_{b}",
                           tag=f"entT{t}_{b}") for b in range(NBLK)]
                for t in range(6)]

        # ---- merged aggregation + projection (Tile interleaves by deps) ----
        HD = CH // 2 * D  # half-block embedding width (5 chunks)
        with tc.tile_pool(name="edges", bufs=2) as edges, \
             tc.tile_pool(name="chunkp", bufs=2) as chunkp, \
             tc.tile_pool(name="evac", bufs=2) as evac, \
             tc.tile_pool(name="wpool", bufs=2) as wpool, \
             tc.tile_pool(name="outp", bufs=2) as outp, \
             tc.tile_pool(name="psagg", bufs=1, space="PSUM") as psagg, \
             tc.tile_pool(name="pp", bufs=2, space="PSUM") as pp:
            for b in range(NBLK):
                halves = []
                for hb in range(2):
                    r0 = b * EPB + hb * (EPB // 2)
                    r1 = r0 + EPB // 2
                    enth = edges.tile([128, HD], bf, tag="enth")
                    nc.gpsimd.dma_start(
                        enth[:],
                        ent_d.ap()[r0:r1, :].rearrange("(p j) d -> p j d", j=CH // 2),
                    )
                    nbrh = edges.tile([128, HD], bf, tag="nbrh")
                    nc.gpsimd.dma_start(
                        nbrh[:],
                        nbr_d.ap()[r0:r1, :].rearrange("(p j) d -> p j d", j=CH // 2),
                    )
                    relh = edges.tile([128, HD], bf, tag="relh")
                    nc.gpsimd.dma_start(
                        relh[:],
                        rel_d.ap()[r0:r1, :].rearrange("(p j) d -> p j d", j=CH // 2),
                    )
                    slh = edges.tile([128, CH // 2], f32, tag="slh")
                    nc.sync.dma_start(
                        slh[:], segl_d.ap()[r0:r1].rearrange("(p j) -> p j", j=CH // 2))
                    cnh = edges.tile([128, CH // 2], f32, tag="cnh")
                    nc.sync.dma_start(
                        cnh[:], cnt_d.ap()[r0:r1].rearrange("(p j) -> p j", j=CH // 2))
                    prh = edges.tile([128, CH // 2], f32, tag="prh")
                    nc.sync.dma_start(
                        prh[:], pr_d.ap()[r0:r1].rearrange("(p j) -> p j", j=CH // 2))
                    halves.append((enth, nbrh, relh, slh, cnh, prh))

                ps_rel = psagg.tile([128, D], f32, tag="ps_rel")
                ps_A = psagg.tile([128, CPAD], f32, tag="ps_A")
                ps_ent = psagg.tile([128, D], f32, tag="ps_ent")

                for j in range(CH):
                    enth, nbrh, relh, slh, cnh, prh = halves[j // 5]
                    jj = j % 5
                    ej = enth[:, jj * D : (jj + 1) * D]
                    nj = nbrh[:, jj * D : (jj + 1) * D]
                    rj = relh[:, jj * D : (jj + 1) * D]
                    scratch = chunkp.tile([128, CPAD], bf, tag="scratch")
                    scr = scratch[:, 0:D]
                    sa = chunkp.tile([128, 1], f32, tag="sa")
                    nc.vector.scalar_tensor_tensor(
                        out=scr, in0=ej, scalar=1.0, in1=wse[:],
                        op0=Alu.mult, op1=Alu.mult, accum_out=sa[:])
                    sb_ = chunkp.tile([128, 1], f32, tag="sb_")
                    nc.vector.scalar_tensor_tensor(
                        out=scr, in0=nj, scalar=1.0, in1=wsn[:],
                        op0=Alu.mult, op1=Alu.mult, accum_out=sb_[:])
                    sc_ = chunkp.tile([128, 1], f32, tag="sc_")
                    nc.vector.scalar_tensor_tensor(
                        out=scr, in0=rj, scalar=1.0, in1=wsr[:],
                        op0=Alu.mult, op1=Alu.mult, accum_out=sc_[:])
                    oc = chunkp.tile([128, CPAD], bf, tag="oc")
                    nc.vector.tensor_scalar(out=oc[:], in0=iota_cnt[:],
                                            scalar1=cnh[:, jj : jj + 1],
                                            scalar2=None, op0=Alu.is_equal)
                    nc.vector.memset(oc[:, CPAD - 1 : CPAD], 1.0)
                    sd_ = chunkp.tile([128, 1], f32, tag="sd_")
                    nc.vector.scalar_tensor_tensor(
                        out=scratch[:], in0=oc[:], scalar=1.0, in1=cscb[:],
                        op0=Alu.mult, op1=Alu.mult, accum_out=sd_[:])
                    op_ = chunkp.tile([128, P], bf, tag="op_")
                    nc.vector.tensor_scalar(out=op_[:], in0=iota_cnt[:, 0:P],
                                            scalar1=prh[:, jj : jj + 1],
                                            scalar2=None, op0=Alu.is_equal)
                    se_ = chunkp.tile([128, 1], f32, tag="se_")
                    nc.vector.scalar_tensor_tensor(
                        out=scratch[:, 0:P], in0=op_[:], scalar=1.0, in1=pscb[:],
                        op0=Alu.mult, op1=Alu.mult, accum_out=se_[:])
                    t1_ = chunkp.tile([128, 1], f32, tag="t1_")
                    nc.vector.tensor_scalar(out=t1_[:], in0=sa[:], scalar1=sb_[:],
                                            scalar2=sc_[:], op0=Alu.add, op1=Alu.add)
                    t2_ = chunkp.tile([128, 1], f32, tag="t2_")
                    nc.vector.tensor_scalar(out=t2_[:], in0=sd_[:], scalar1=se_[:],
                                            scalar2=None, op0=Alu.add)
                    ex_ = chunkp.tile([128, 1], f32, tag="ex_")
                    nc.scalar.activation(ex_[:], t1_[:], Act.Exp, bias=t2_[:])
                    oh = chunkp.tile([128, 128], bf, tag="oh")
                    nc.vector.tensor_scalar(out=oh[:], in0=iota_seg[:],
                                            scalar1=slh[:, jj : jj + 1],
                                            scalar2=None, op0=Alu.is_equal)
                    ohx = chunkp.tile([128, 128], bf, tag="ohx")
                    nc.vector.tensor_scalar(out=ohx[:], in0=iota_seg[:],
                                            scalar1=slh[:, jj : jj + 1],
                                            scalar2=ex_[:],
                                            op0=Alu.is_equal, op1=Alu.mult)
                    st, sp = (j == 0), (j == CH - 1)
                    nc.tensor.matmul(ps_rel[:, 0:512], ohx[:], rj[:, 0:512],
                                     start=st, stop=sp)
                    nc.tensor.matmul(ps_rel[:, 512:D], ohx[:], rj[:, 512:D],
                                     start=st, stop=sp)
                    nc.tensor.matmul(ps_A[:, 0:512], ohx[:], oc[:, 0:512],
                                     start=st, stop=sp)
                    nc.tensor.matmul(ps_A[:, 512:CPAD], ohx[:], oc[:, 512:CPAD],
                                     start=st, stop=sp)
                    nc.tensor.matmul(ps_ent[:, 0:512], oh[:], ej[:, 0:512],
                                     start=st, stop=sp)
                    nc.tensor.matmul(ps_ent[:, 512:D], oh[:], ej[:, 512:D],
                                     start=st, stop=sp)

                # block epilogue
                dmx = chunkp.tile([128, 1], f32, tag="dmx")
                nc.vector.tensor_scalar(out=dmx[:], in0=ps_A[:, CPAD - 1 : CPAD],
                                        scalar1=1e-30, scalar2=None, op0=Alu.max)
                nc.vector.reciprocal(invd_sb[:, b : b + 1], dmx[:])
                relsb = evac.tile([128, D], bf, tag="relsb")
                nc.scalar.activation(relsb[:], ps_rel[:], Act.Copy,
                                     scale=invd_sb[:, b : b + 1])
                Asb = evac.tile([128, CPAD], bf, tag="Asb")
                nc.scalar.activation(Asb[:], ps_A[:], Act.Copy,
                                     scale=invd_sb[:, b : b + 1])
                entsb = evac.tile([128, D], bf, tag="entsb")
                nc.scalar.activation(entsb[:], ps_ent[:], Act.Copy,
                                     scale=icnt_sb[:, b : b + 1])

                bs = slice(b * 128, (b + 1) * 128)
                for t in range(6):
                    pt = pp.tile([128, 512], bf, tag="pp")
                    nc.tensor.transpose(pt[:, 0:128], relsb[:, t * 128 : (t + 1) * 128],
                                        ident[:])
                    nc.scalar.activation(relcatT[t][b][:], pt[:, 0:128], Act.Copy)
                    pt2 = pp.tile([128, 512], bf, tag="pp")
                    nc.tensor.transpose(pt2[:, 0:128], entsb[:, t * 128 : (t + 1) * 128],
                                        ident[:])
                    nc.scalar.activation(entT[t][b][:], pt2[:, 0:128], Act.Copy)
                ATl = []
                for t in range(8):
                    pt3 = pp.tile([128, 512], bf, tag="pp")
                    nc.tensor.transpose(pt3[:, 0:128], Asb[:, t * 128 : (t + 1) * 128],
                                        ident[:])
                    at = evac.tile([128, 128], bf, name=f"AT{t}", tag=f"AT{t}")
                    nc.scalar.activation(at[:], pt3[:, 0:128], Act.Copy)
                    ATl.append(at)
                for dchunk in range(6):
                    pc = pp.tile([128, 512], f32, tag="pp")
                    for cc in range(8):
                        nc.tensor.matmul(
                            pc[:, 0:128],
                            ctsb[:, cc * D + dchunk * 128 : cc * D + (dchunk + 1) * 128],
                            ATl[cc][:],
                            start=(cc == 0), stop=(cc == 7))
                    nc.scalar.activation(relcatT[6 + dchunk][b][:], pc[:, 0:128],
                                         Act.Copy)

            # ---- projections (interleave with later aggregation blocks) ----
            for (Tt, wt_d, b_d, o_d, KC) in (
                (relcatT, wtr_d, brel_d, orel_d, 12),
                (entT, wte_d, bent_d, oent_d, 6),
            ):
                for h in range(5):
                    wt = wpool.tile([128, KC * OH], bf, tag="wt")
                    for k in range(KC):
                        nc.gpsimd.dma_start(
                            wt[:, k * OH : (k + 1) * OH],
                            wt_d.ap()[k * 128 : (k + 1) * 128,
                                      h * OH : (h + 1) * OH],
                        )
                    bt = wpool.tile([1, OH], bf, tag="bt")
                    nc.gpsimd.dma_start(
                        bt[:],
                        b_d.ap()[h * OH : (h + 1) * OH].rearrange(
                            "(o c) -> o c", o=1),
                    )
                    for sblk in range(NBLK):
                        stage = outp.tile([128, OH], f32, tag="stage")
                        for oc5 in range(OH // 512):
                            pso = pp.tile([128, 512], f32, tag="pp")
                            nc.tensor.matmul(pso[:], ones_r[:],
                                             bt[:, oc5 * 512 : (oc5 + 1) * 512],
                                             start=True, stop=False)
                            for k in range(KC):
                                nc.tensor.matmul(
                                    pso[:],
                                    Tt[k][sblk][:],
                                    wt[:, k * OH + oc5 * 512 : k * OH + (oc5 + 1) * 512],
                                    start=False, stop=(k == KC - 1))
                            if oc5 % 2 == 0:
                                nc.vector.tensor_copy(
                                    stage[:, oc5 * 512 : (oc5 + 1) * 512], pso[:])
                            else:
                                nc.scalar.activation(
                                    stage[:, oc5 * 512 : (oc5 + 1) * 512], pso[:],
                                    Act.Copy)
                        nc.sync.dma_start(
                            o_d.ap()[sblk * 128 : (sblk + 1) * 128,
                                     h * OH : (h + 1) * OH],
                            stage[:],
                        )
    return nc


_NC_CACHE = None


def _get_nc():
    global _NC_CACHE
    if _NC_CACHE is None:
        _NC_CACHE = _build_nc()
    return _NC_CACHE


# --------------------------------------------------------------------------
# entry point
# --------------------------------------------------------------------------

def kernel(prompt_embs, entity_embs, neighbor_embs, relation_embs,
           count_table, scorer_W, scorer_b, rel_W, rel_b, ent_W, ent_b,
           counts, prompt_indices, entity_indices):
    from concourse.bass_utils import run_bass_kernel_spmd

    prompt_embs = np.asarray(prompt_embs, dtype=np.float32)
    entity_embs = np.asarray(entity_embs, dtype=np.float32)
    neighbor_embs = np.asarray(neighbor_embs, dtype=np.float32)
    relation_embs = np.asarray(relation_embs, dtype=np.float32)
    count_table = np.asarray(count_table, dtype=np.float32)
    scorer_W = np.asarray(scorer_W, dtype=np.float32)
    scorer_b = np.asarray(scorer_b, dtype=np.float32)
    rel_W = np.asarray(rel_W, dtype=np.float32)
    rel_b = np.asarray(rel_b, dtype=np.float32)
    ent_W = np.asarray(ent_W, dtype=np.float32)
    ent_b = np.asarray(ent_b, dtype=np.float32)
    counts = np.asarray(counts)
    prompt_indices = np.asarray(prompt_indices)
    entity_indices = np.asarray(entity_indices)

    cores = _shard_and_pack(entity_indices)

    # replicated (weight-derived) host prep
    w = scorer_W[0]
    w1, w2, w3, w4, w5 = (w[i * D : (i + 1) * D] for i in range(5))
    pscore = (prompt_embs * w1[None, :]).sum(1) + scorer_b[0]     # fold bias
    cscore = (count_table * w5[None, :]).sum(1)
    cs_pad = np.zeros(CPAD, np.float32)
    cs_pad[:C] = cscore
    cscb = np.broadcast_to(cs_pad.astype(BF16), (128, CPAD)).copy()
    pscb = np.broadcast_to(pscore.astype(BF16), (128, P)).copy()
    wse = np.broadcast_to(w2.astype(BF16), (128, D)).copy()
    wsn = np.broadcast_to(w3.astype(BF16), (128, D)).copy()
    wsr = np.broadcast_to(w4.astype(BF16), (128, D)).copy()
    ctp = np.zeros((CPAD, D), np.float32)
    ctp[:C] = count_table
    wtr = np.ascontiguousarray(rel_W.T)     # [2D, OUT]
    wte = np.ascontiguousarray(ent_W.T)     # [D, OUT]

    in_maps = []
    for core in cores:
        perm = core["perm"]
        valid = perm >= 0
        src = np.where(valid, perm, 0)

        def take2d(a):
            out = a[src]
            out[~valid] = 0.0
            return np.ascontiguousarray(out)

        def take1d(a):
            out = a.astype(np.float32)[src]
            out[~valid] = 0.0
            return np.ascontiguousarray(out)

        in_maps.append(dict(
            ent=take2d(entity_embs), nbr=take2d(neighbor_embs),
            rel=take2d(relation_embs),
            segl=core["seg_local"], cntf=take1d(counts),
            prf=take1d(prompt_indices), inv_cnt=core["inv_cnt"],
            cscb=cscb, pscb=pscb, wse=wse, wsn=wsn, wsr=wsr,
            ctp=ctp, wtr=wtr, wte=wte, brel=rel_b, bent=ent_b,
        ))

    nc = _get_nc()
    res = run_bass_kernel_spmd(nc, in_maps, list(range(N_CORES)))

    rel_out = np.zeros((E, OUT), np.float32)
    ent_out = np.zeros((E, OUT), np.float32)
    for c, core in enumerate(cores):
        rows = core["row2seg"]
        mask = rows >= 0
        rel_out[rows[mask]] = res.results[c]["orel"][mask]
        ent_out[rows[mask]] = res.results[c]["oent"][mask]
    return rel_out, ent_out



# revision 9
# speedup vs baseline: 1.4257x; 1.4257x over previous
"""EntityEncoder (gnn_message_passing) Trainium2 kernel — 8-core SPMD, v2.

Strategy: edges pre-partitioned on host into 8 contiguous entity-aligned
shards (entity_indices sorted => no cross-core collectives). Per core,
segments LPT-packed into 10 blocks of <=128 segments / <=1280 edges.

v2 changes vs v1:
  - all embedding streams converted to bf16 on host (halves HBM reads)
  - count embeddings gathered on host into a 4th edge stream (removes
    one-hot count/prompt vector work and the count-table matmul chain)
  - prompt/count scorer contributions folded on host into one per-edge
    scalar (exp bias)
  - two-phase device schedule: aggregation (one-hot matmuls + PE
    transposes into resident transposed aggregates), then projection
    with weight-stationary matmuls (one LDW per 1280 streamed cols)
  - outputs written transposed [OUT, E_PAD] in bf16; bias + transpose
    + scatter on host
"""
import sys
import numpy as np
import ml_dtypes

for _p in ("/root/.axon_site", "/root/.axon_site/_ro/trn_rl_repo",
           "/root/.axon_site/_ro/pypackages"):
    if _p not in sys.path:
        sys.path.append(_p)

import bass_rust
import concourse.bass as bass
import concourse.mybir as mybir
import concourse.tile as tile
from concourse.vector_clock import ScopedClock
from contextlib import ExitStack

BF16 = ml_dtypes.bfloat16
dt = mybir.dt
Alu = mybir.AluOpType
Act = mybir.ActivationFunctionType

# problem shape (hardcoded per contest contract)
N_CORES = 8
N = 100_000
P = 64
E = 10_000
D = 768
C = 1000
OUT = 5120
# per-core packing
NBLK = 10
SPB = 128                # segs per block
CH = 10                  # chunks (of 128 edges) per block
EPB = CH * 128           # edges per block = 1280
NL = NBLK * EPB          # 12800 edge slots per core
E_PAD = NBLK * SPB       # 1280 seg slots per core
KC_R = 12                # rel|cnt contraction chunks (1536/128)
KC_E = 6                 # ent contraction chunks (768/128)
DC = 832                 # cnt stream width: 768 emb + ones col at 768
NOT = OUT // 128         # 40 output tiles of 128 cols
PAD_SEG = 999.0


class _TileContextSplitDrain(tile.TileContext):
    """This container's walrus accepts only ONE sync wait per instruction
    ("Too many sync wait commands" in setupSyncWait). Split every extra wait
    onto a standalone same-engine NoOp placed immediately before the
    instruction — identical semantics, one wait per instruction."""

    def _lower_ordered_insts(self, ordered):
        for insts in ordered.values():
            if not any(
                i.sync_info is not None and len(i.sync_info.on_wait) > 1
                for i in insts
            ):
                continue
            new = []
            for inst in insts:
                si = inst.sync_info
                if si is not None and len(si.on_wait) > 1:
                    waits = list(si.on_wait)
                    for w in waits[:-1]:
                        nop = bass_rust.InstNoOp(
                            name=self.nc.get_next_instruction_name(),
                            ins=[], outs=[])
                        nop.engine = inst.engine
                        nop.sync_info = bass_rust.SyncInfo(
                            on_wait=[w], on_update=[])
                        new.append(nop)
                    si.on_wait = waits[-1:]
                new.append(inst)
            insts[:] = new
        return super()._lower_ordered_insts(ordered)

    def _drain_and_barrier(self, tick_clock, wait_clock):
        nc = self.nc
        drain_inst = nc.sync.drain()
        wait_clock.add_sem_waits(
            drain_inst.ins, ScopedClock({None: tick_clock.global_clock})
        )
        si = drain_inst.ins.sync_info
        if si is not None and len(si.on_wait) > 1:
            waits = list(si.on_wait)
            si.on_wait = waits[:1]
            for w in waits[1:]:
                n = nc.sync.nop()
                n.ins.sync_info = bass_rust.SyncInfo(on_wait=[w], on_update=[])
        nc.all_engine_barrier()
        assert self.sems is not None
        popped = nc._tile_sem_poison_stack.pop()
        assert popped is self._sem_poison
        nc.clear_and_free_semaphores(list(self.sems.allocated().values()))
        nc.all_engine_barrier()


# --------------------------------------------------------------------------
# host-side sharding / packing
# --------------------------------------------------------------------------

def _shard_and_pack(entity_indices):
    Nn = entity_indices.shape[0]
    starts = np.searchsorted(entity_indices, np.arange(E + 1))
    ideal = (np.arange(1, N_CORES) * Nn) // N_CORES
    ent_bnd = [0]
    for t in ideal:
        s = int(np.searchsorted(starts, t))
        if s > 0 and abs(int(starts[s - 1]) - int(t)) < abs(int(starts[s]) - int(t)):
            s -= 1
        ent_bnd.append(s)
    ent_bnd.append(E)

    cores = []
    for c in range(N_CORES):
        e_lo, e_hi = ent_bnd[c], ent_bnd[c + 1]
        segs = np.arange(e_lo, e_hi)
        sizes = (starts[e_lo + 1 : e_hi + 1] - starts[e_lo:e_hi]).astype(np.int64)
        n_edges = int(sizes.sum())
        assert e_hi - e_lo <= E_PAD and n_edges <= NL
        order = np.argsort(-sizes, kind="stable")
        blk_edges = [0] * NBLK
        blk_nseg = [0] * NBLK
        blk_segs = [[] for _ in range(NBLK)]
        for idx in order:
            sz = int(sizes[idx])
            best = -1
            for b in sorted(range(NBLK), key=lambda b: blk_edges[b]):
                if blk_nseg[b] < SPB and blk_edges[b] + sz <= EPB:
                    best = b
                    break
            assert best >= 0, "block packing overflow"
            blk_segs[best].append(int(segs[idx]))
            blk_edges[best] += sz
            blk_nseg[best] += 1
        perm = np.full(NL, -1, dtype=np.int64)
        seg_local = np.full(NL, PAD_SEG, dtype=np.float32)
        row2seg = np.full(E_PAD, -1, dtype=np.int64)
        inv_cnt = np.zeros(E_PAD, dtype=np.float32)
        for b in range(NBLK):
            pos = b * EPB
            for j, s in enumerate(blk_segs[b]):
                row = b * SPB + j
                row2seg[row] = s
                n = int(starts[s + 1] - starts[s])
                if n > 0:
                    inv_cnt[row] = 1.0 / n
                perm[pos : pos + n] = np.arange(starts[s], starts[s + 1])
                seg_local[pos : pos + n] = float(j)
                pos += n
        cores.append(dict(perm=perm, seg_local=seg_local, row2seg=row2seg,
                          inv_cnt=inv_cnt))
    return cores


# --------------------------------------------------------------------------
# device kernel
# --------------------------------------------------------------------------

def _build_nc():
    nc = bass.Bass("TRN2", target_bir_lowering=False, debug=False,
                   num_devices=N_CORES)

    f32, bf, i32 = dt.float32, dt.bfloat16, dt.int32
    din = lambda n, s, d=f32: nc.dram_tensor(n, s, d, kind="ExternalInput")
    ent_d = din("ent", [NL, D], bf)
    nbr_d = din("nbr", [NL, D], bf)
    rel_d = din("rel", [NL, D], bf)
    cnt_d = din("cnt", [NL, DC], bf)
    segl_d = din("segl", [NL])
    sc0_d = din("sc0", [NL])
    icnt_d = din("inv_cnt", [E_PAD])
    wse_d = din("wse", [128, D], bf)
    wsn_d = din("wsn", [128, D], bf)
    wsr_d = din("wsr", [128, D], bf)
    # tiled projector weights: [ot, k, 128, 128] (lhsT tiles)
    wtr_d = din("wtr", [NOT, KC_R, 128, 128], bf)
    wte_d = din("wte", [NOT, KC_E, 128, 128], bf)
    orelT_d = nc.dram_tensor("orelT", [OUT, E_PAD], bf, kind="ExternalOutput")
    oentT_d = nc.dram_tensor("oentT", [OUT, E_PAD], bf, kind="ExternalOutput")

    with _TileContextSplitDrain(nc) as tc, ExitStack() as es:
        const = es.enter_context(tc.tile_pool(name="const", bufs=1))
        accp = es.enter_context(tc.tile_pool(name="accp", bufs=1))

        # ---- constants ----
        iota_seg = const.tile([128, 128], bf)
        ident = const.tile([128, 128], bf)
        with tc.tile_pool(name="setup", bufs=1) as setup:
            iota_i = setup.tile([128, 128], i32)
            nc.gpsimd.iota(iota_i[:], pattern=[[1, 128]], base=0,
                           channel_multiplier=0)
            nc.vector.tensor_copy(iota_seg[:], iota_i[:])
            iota_ci = setup.tile([128, 1], i32)
            nc.gpsimd.iota(iota_ci[:], pattern=[[0, 1]], base=0,
                           channel_multiplier=1)
            iota_col = setup.tile([128, 1], f32)
            nc.vector.tensor_copy(iota_col[:], iota_ci[:])
            nc.vector.tensor_scalar(out=ident[:], in0=iota_seg[:],
                                    scalar1=iota_col[:],
                                    scalar2=None, op0=Alu.is_equal)
        wse = const.tile([128, D], bf)
        nc.sync.dma_start(wse[:], wse_d.ap())
        wsn = const.tile([128, D], bf)
        nc.sync.dma_start(wsn[:], wsn_d.ap())
        wsr = const.tile([128, D], bf)
        nc.sync.dma_start(wsr[:], wsr_d.ap())
        icnt_sb = const.tile([128, NBLK], f32)
        nc.sync.dma_start(
            icnt_sb[:], icnt_d.ap().rearrange("(b p) -> p b", p=128)
        )
        invd_sb = accp.tile([128, NBLK], f32)

        # resident transposed aggregates: ATr[k] = [128 (k-dim), 1280 (seg)]
        ATr = [accp.tile([128, E_PAD], bf, name=f"ATr{k}", tag=f"ATr{k}")
               for k in range(KC_R)]
        ATe = [accp.tile([128, E_PAD], bf, name=f"ATe{k}", tag=f"ATe{k}")
               for k in range(KC_E)]

        HD = (CH // 2) * D  # half-block embedding width (5 chunks)

        # ================= Phase A: aggregation =================
        with tc.tile_pool(name="edges", bufs=3) as edges, \
             tc.tile_pool(name="chunkp", bufs=4) as chunkp, \
             tc.tile_pool(name="scrp", bufs=2) as scrp, \
             tc.tile_pool(name="evac", bufs=2) as evac, \
             tc.tile_pool(name="psagg", bufs=1, space="PSUM") as psagg, \
             tc.tile_pool(name="pp", bufs=2, space="PSUM") as pp:
            for b in range(NBLK):
                halves = []
                for hb in range(2):
                    r0 = b * EPB + hb * (EPB // 2)
                    r1 = r0 + EPB // 2
                    enth = edges.tile([128, HD], bf, tag="enth")
                    nc.gpsimd.dma_start(
                        enth[:],
                        ent_d.ap()[r0:r1, :].rearrange("(p j) d -> p j d", j=CH // 2),
                    )
                    nbrh = edges.tile([128, HD], bf, tag="nbrh")
                    nc.gpsimd.dma_start(
                        nbrh[:],
                        nbr_d.ap()[r0:r1, :].rearrange("(p j) d -> p j d", j=CH // 2),
                    )
                    relh = edges.tile([128, HD], bf, tag="relh")
                    nc.gpsimd.dma_start(
                        relh[:],
                        rel_d.ap()[r0:r1, :].rearrange("(p j) d -> p j d", j=CH // 2),
                    )
                    cnth = edges.tile([128, (CH // 2) * DC], bf, tag="cnth")
                    nc.gpsimd.dma_start(
                        cnth[:],
                        cnt_d.ap()[r0:r1, :].rearrange("(p j) d -> p j d", j=CH // 2),
                    )
                    slh = edges.tile([128, CH // 2], f32, tag="slh")
                    nc.sync.dma_start(
                        slh[:], segl_d.ap()[r0:r1].rearrange("(p j) -> p j", j=CH // 2))
                    sch = edges.tile([128, CH // 2], f32, tag="sch")
                    nc.sync.dma_start(
                        sch[:], sc0_d.ap()[r0:r1].rearrange("(p j) -> p j", j=CH // 2))
                    halves.append((enth, nbrh, relh, cnth, slh, sch))

                ps_rc = psagg.tile([128, 2048], f32, tag="ps_rc")
                ps_ed = psagg.tile([128, 1024], f32, tag="ps_ed")

                for j in range(CH):
                    enth, nbrh, relh, cnth, slh, sch = halves[j // 5]
                    jj = j % 5
                    ej = enth[:, jj * D : (jj + 1) * D]
                    nj = nbrh[:, jj * D : (jj + 1) * D]
                    rj = relh[:, jj * D : (jj + 1) * D]
                    cj = cnth[:, jj * DC : jj * DC + DC]
                    scr = scrp.tile([128, D], bf, tag="scr")
                    sa = chunkp.tile([128, 1], f32, tag="sa")
                    nc.vector.scalar_tensor_tensor(
                        out=scr[:], in0=ej, scalar=1.0, in1=wse[:],
                        op0=Alu.mult, op1=Alu.mult, accum_out=sa[:])
                    sb_ = chunkp.tile([128, 1], f32, tag="sb_")
                    nc.vector.scalar_tensor_tensor(
                        out=scr[:], in0=nj, scalar=1.0, in1=wsn[:],
                        op0=Alu.mult, op1=Alu.mult, accum_out=sb_[:])
                    sc_ = chunkp.tile([128, 1], f32, tag="sc_")
                    nc.vector.scalar_tensor_tensor(
                        out=scr[:], in0=rj, scalar=1.0, in1=wsr[:],
                        op0=Alu.mult, op1=Alu.mult, accum_out=sc_[:])
                    t1_ = chunkp.tile([128, 1], f32, tag="t1_")
                    nc.vector.tensor_scalar(out=t1_[:], in0=sa[:], scalar1=sb_[:],
                                            scalar2=sc_[:], op0=Alu.add, op1=Alu.add)
                    ex_ = chunkp.tile([128, 1], f32, tag="ex_")
                    nc.scalar.activation(ex_[:], t1_[:], Act.Exp,
                                         bias=sch[:, jj : jj + 1])
                    oh = chunkp.tile([128, 128], bf, tag="oh")
                    nc.vector.tensor_scalar(out=oh[:], in0=iota_seg[:],
                                            scalar1=slh[:, jj : jj + 1],
                                            scalar2=None, op0=Alu.is_equal)
                    ohx = chunkp.tile([128, 128], bf, tag="ohx")
                    nc.vector.tensor_scalar(out=ohx[:], in0=iota_seg[:],
                                            scalar1=slh[:, jj : jj + 1],
                                            scalar2=ex_[:],
                                            op0=Alu.is_equal, op1=Alu.mult)
                    st, sp = (j == 0), (j == CH - 1)
                    nc.tensor.matmul(ps_rc[:, 0:512], ohx[:], rj[:, 0:512],
                                     start=st, stop=sp)
                    nc.tensor.matmul(ps_rc[:, 512:768], ohx[:], rj[:, 512:768],
                                     start=st, stop=sp)
                    nc.tensor.matmul(ps_rc[:, 1024:1536], ohx[:], cj[:, 0:512],
                                     start=st, stop=sp)
                    nc.tensor.matmul(ps_rc[:, 1536:1856], ohx[:], cj[:, 512:832],
                                     start=st, stop=sp)
                    nc.tensor.matmul(ps_ed[:, 0:512], oh[:], ej[:, 0:512],
                                     start=st, stop=sp)
                    nc.tensor.matmul(ps_ed[:, 512:768], oh[:], ej[:, 512:768],
                                     start=st, stop=sp)

                # block epilogue: invd, normalized bf16 evacs, transposes
                dmx = chunkp.tile([128, 1], f32, tag="dmx")
                nc.vector.tensor_scalar(out=dmx[:], in0=ps_rc[:, 1792:1793],
                                        scalar1=1e-30, scalar2=None, op0=Alu.max)
                nc.vector.reciprocal(invd_sb[:, b : b + 1], dmx[:])
                rcsb = evac.tile([128, 2 * D], bf, tag="rcsb")
                nc.scalar.activation(rcsb[:, 0:768], ps_rc[:, 0:768], Act.Copy,
                                     scale=invd_sb[:, b : b + 1])
                nc.scalar.activation(rcsb[:, 768:1536], ps_rc[:, 1024:1792],
                                     Act.Copy, scale=invd_sb[:, b : b + 1])
                edsb = evac.tile([128, D], bf, tag="edsb")
                nc.scalar.activation(edsb[:], ps_ed[:, 0:768], Act.Copy,
                                     scale=icnt_sb[:, b : b + 1])

                bs = slice(b * 128, (b + 1) * 128)
                for k in range(KC_R):
                    pt = pp.tile([128, 512], bf, tag="pp")
                    nc.tensor.transpose(pt[:, 0:128],
                                        rcsb[:, k * 128 : (k + 1) * 128],
                                        ident[:])
                    if k % 2 == 0:
                        nc.vector.tensor_copy(ATr[k][:, bs], pt[:, 0:128])
                    else:
                        nc.scalar.activation(ATr[k][:, bs], pt[:, 0:128],
                                             Act.Copy)
                for k in range(KC_E):
                    pt = pp.tile([128, 512], bf, tag="pp")
                    nc.tensor.transpose(pt[:, 0:128],
                                        edsb[:, k * 128 : (k + 1) * 128],
                                        ident[:])
                    if k % 2 == 0:
                        nc.vector.tensor_copy(ATe[k][:, bs], pt[:, 0:128])
                    else:
                        nc.scalar.activation(ATe[k][:, bs], pt[:, 0:128],
                                             Act.Copy)

        # ================= Phase B: projections =================
        with tc.tile_pool(name="wpool", bufs=3) as wpool, \
             tc.tile_pool(name="outp", bufs=3) as outp, \
             tc.tile_pool(name="ppb", bufs=2, space="PSUM") as ppb:
            for part, (wt_d, ATl, KC, o_d) in enumerate((
                (wtr_d, ATr, KC_R, orelT_d),
                (wte_d, ATe, KC_E, oentT_d),
            )):
                for ot in range(NOT):
                    wt = wpool.tile([128, KC * 128], bf, tag=f"wt{part}")
                    nc.gpsimd.dma_start(
                        wt[:],
                        wt_d.ap()[ot].rearrange("k p c -> p k c"),
                    )
                    ps = ppb.tile([128, E_PAD], f32, tag="ppb")
                    for k in range(KC):
                        lw = wt[:, k * 128 : (k + 1) * 128]
                        st, sp = (k == 0), (k == KC - 1)
                        nc.tensor.matmul(ps[:, 0:512], lw, ATl[k][:, 0:512],
                                         start=st, stop=sp)
                        nc.tensor.matmul(ps[:, 512:1024], lw, ATl[k][:, 512:1024],
                                         start=st, stop=sp)
                        nc.tensor.matmul(ps[:, 1024:1280], lw, ATl[k][:, 1024:1280],
                                         start=st, stop=sp)
                    stage = outp.tile([128, E_PAD], bf, tag="stage")
                    if ot % 2 == 0:
                        nc.vector.tensor_copy(stage[:], ps[:])
                    else:
                        nc.scalar.activation(stage[:], ps[:], Act.Copy)
                    nc.sync.dma_start(
                        o_d.ap()[ot * 128 : (ot + 1) * 128, :], stage[:]
                    )
    return nc


_NC_CACHE = None


def _get_nc():
    global _NC_CACHE
    if _NC_CACHE is None:
        _NC_CACHE = _build_nc()
    return _NC_CACHE


# --------------------------------------------------------------------------
# entry point
# --------------------------------------------------------------------------

def kernel(prompt_embs, entity_embs, neighbor_embs, relation_embs,
           count_table, scorer_W, scorer_b, rel_W, rel_b, ent_W, ent_b,
           counts, prompt_indices, entity_indices):
    from concourse.bass_utils import run_bass_kernel_spmd

    prompt_embs = np.asarray(prompt_embs, dtype=np.float32)
    entity_embs = np.asarray(entity_embs, dtype=np.float32)
    neighbor_embs = np.asarray(neighbor_embs, dtype=np.float32)
    relation_embs = np.asarray(relation_embs, dtype=np.float32)
    count_table = np.asarray(count_table, dtype=np.float32)
    scorer_W = np.asarray(scorer_W, dtype=np.float32)
    scorer_b = np.asarray(scorer_b, dtype=np.float32)
    rel_W = np.asarray(rel_W, dtype=np.float32)
    rel_b = np.asarray(rel_b, dtype=np.float32)
    ent_W = np.asarray(ent_W, dtype=np.float32)
    ent_b = np.asarray(ent_b, dtype=np.float32)
    counts = np.asarray(counts)
    prompt_indices = np.asarray(prompt_indices)
    entity_indices = np.asarray(entity_indices)

    cores = _shard_and_pack(entity_indices)

    # replicated (weight-derived) host prep
    w = scorer_W[0]
    w1, w2, w3, w4, w5 = (w[i * D : (i + 1) * D] for i in range(5))
    pscore = (prompt_embs * w1[None, :]).sum(1) + scorer_b[0]     # fold bias
    cscore = (count_table * w5[None, :]).sum(1)
    wse = np.broadcast_to(w2.astype(BF16), (128, D)).copy()
    wsn = np.broadcast_to(w3.astype(BF16), (128, D)).copy()
    wsr = np.broadcast_to(w4.astype(BF16), (128, D)).copy()
    # tiled lhsT weight tiles: wtr[ot, k, kl, ol] = rel_W[ot*128+ol, k*128+kl]
    wtr = np.ascontiguousarray(
        rel_W.T.reshape(KC_R, 128, NOT, 128).transpose(2, 0, 1, 3)
    ).astype(BF16)
    wte = np.ascontiguousarray(
        ent_W.T.reshape(KC_E, 128, NOT, 128).transpose(2, 0, 1, 3)
    ).astype(BF16)

    ent16 = entity_embs.astype(BF16)
    nbr16 = neighbor_embs.astype(BF16)
    rel16 = relation_embs.astype(BF16)
    cnt16 = np.zeros((N, DC), dtype=BF16)
    cnt16[:, 0:D] = count_table.astype(BF16)[counts]  # [N, D] gather
    cnt16[:, D] = BF16(1.0)                           # denominator ones col
    sc0_full = (pscore[prompt_indices] + cscore[counts]).astype(np.float32)

    in_maps = []
    for core in cores:
        perm = core["perm"]
        valid = perm >= 0
        src = np.where(valid, perm, 0)

        def take2d(a16):
            out = a16[src]
            out[~valid] = 0.0
            return np.ascontiguousarray(out)

        sc0 = sc0_full[src]
        sc0[~valid] = 0.0

        in_maps.append(dict(
            ent=take2d(ent16), nbr=take2d(nbr16),
            rel=take2d(rel16), cnt=take2d(cnt16),
            segl=core["seg_local"], sc0=np.ascontiguousarray(sc0),
            inv_cnt=core["inv_cnt"],
            wse=wse, wsn=wsn, wsr=wsr, wtr=wtr, wte=wte,
        ))

    nc = _get_nc()
    res = run_bass_kernel_spmd(nc, in_maps, list(range(N_CORES)))

    rel_out = np.zeros((E, OUT), np.float32)
    ent_out = np.zeros((E, OUT), np.float32)
    for c, core in enumerate(cores):
        rows = core["row2seg"]
        mask = rows >= 0
        rel_out[rows[mask]] = res.results[c]["orelT"].T[mask].astype(np.float32)
        ent_out[rows[mask]] = res.results[c]["oentT"].T[mask].astype(np.float32)
    rel_out += rel_b[None, :]
    ent_out += ent_b[None, :]
    return rel_out, ent_out


# revision 12
# speedup vs baseline: 1.4757x; 1.0351x over previous
"""EntityEncoder (gnn_message_passing) Trainium2 kernel — 8-core SPMD, v2.

Strategy: edges pre-partitioned on host into 8 contiguous entity-aligned
shards (entity_indices sorted => no cross-core collectives). Per core,
segments LPT-packed into 10 blocks of <=128 segments / <=1280 edges.

v2 changes vs v1:
  - all embedding streams converted to bf16 on host (halves HBM reads)
  - count embeddings gathered on host into a 4th edge stream (removes
    one-hot count/prompt vector work and the count-table matmul chain)
  - prompt/count scorer contributions folded on host into one per-edge
    scalar (exp bias)
  - two-phase device schedule: aggregation (one-hot matmuls + PE
    transposes into resident transposed aggregates), then projection
    with weight-stationary matmuls (one LDW per 1280 streamed cols)
  - outputs written transposed [OUT, E_PAD] in bf16; bias + transpose
    + scatter on host
"""
import sys
import numpy as np
import ml_dtypes

for _p in ("/root/.axon_site", "/root/.axon_site/_ro/trn_rl_repo",
           "/root/.axon_site/_ro/pypackages"):
    if _p not in sys.path:
        sys.path.append(_p)

import bass_rust
import concourse.bass as bass
import concourse.mybir as mybir
import concourse.tile as tile
from concourse.vector_clock import ScopedClock
from contextlib import ExitStack

BF16 = ml_dtypes.bfloat16
dt = mybir.dt
Alu = mybir.AluOpType
Act = mybir.ActivationFunctionType

# problem shape (hardcoded per contest contract)
N_CORES = 8
N = 100_000
P = 64
E = 10_000
D = 768
C = 1000
OUT = 5120
# per-core packing
NBLK = 10
SPB = 128                # segs per block
CH = 10                  # chunks (of 128 edges) per block
EPB = CH * 128           # edges per block = 1280
NL = NBLK * EPB          # 12800 edge slots per core
E_PAD = NBLK * SPB       # 1280 seg slots per core
KC_R = 12                # rel|cnt contraction chunks (1536/128)
KC_E = 6                 # ent contraction chunks (768/128)
DC = 832                 # cnt stream width: 768 emb + ones col at 768
NOT = OUT // 128         # 40 output tiles of 128 cols
PAD_SEG = 999.0


class _TileContextSplitDrain(tile.TileContext):
    """This container's walrus accepts only ONE sync wait per instruction
    ("Too many sync wait commands" in setupSyncWait). Split every extra wait
    onto a standalone same-engine NoOp placed immediately before the
    instruction — identical semantics, one wait per instruction."""

    def _lower_ordered_insts(self, ordered):
        for insts in ordered.values():
            if not any(
                i.sync_info is not None and len(i.sync_info.on_wait) > 1
                for i in insts
            ):
                continue
            new = []
            for inst in insts:
                si = inst.sync_info
                if si is not None and len(si.on_wait) > 1:
                    waits = list(si.on_wait)
                    for w in waits[:-1]:
                        nop = bass_rust.InstNoOp(
                            name=self.nc.get_next_instruction_name(),
                            ins=[], outs=[])
                        nop.engine = inst.engine
                        nop.sync_info = bass_rust.SyncInfo(
                            on_wait=[w], on_update=[])
                        new.append(nop)
                    si.on_wait = waits[-1:]
                new.append(inst)
            insts[:] = new
        return super()._lower_ordered_insts(ordered)

    def _drain_and_barrier(self, tick_clock, wait_clock):
        nc = self.nc
        drain_inst = nc.sync.drain()
        wait_clock.add_sem_waits(
            drain_inst.ins, ScopedClock({None: tick_clock.global_clock})
        )
        si = drain_inst.ins.sync_info
        if si is not None and len(si.on_wait) > 1:
            waits = list(si.on_wait)
            si.on_wait = waits[:1]
            for w in waits[1:]:
                n = nc.sync.nop()
                n.ins.sync_info = bass_rust.SyncInfo(on_wait=[w], on_update=[])
        nc.all_engine_barrier()
        assert self.sems is not None
        popped = nc._tile_sem_poison_stack.pop()
        assert popped is self._sem_poison
        nc.clear_and_free_semaphores(list(self.sems.allocated().values()))
        nc.all_engine_barrier()


# --------------------------------------------------------------------------
# host-side sharding / packing
# --------------------------------------------------------------------------

def _shard_and_pack(entity_indices):
    Nn = entity_indices.shape[0]
    starts = np.searchsorted(entity_indices, np.arange(E + 1))
    ideal = (np.arange(1, N_CORES) * Nn) // N_CORES
    ent_bnd = [0]
    for t in ideal:
        s = int(np.searchsorted(starts, t))
        if s > 0 and abs(int(starts[s - 1]) - int(t)) < abs(int(starts[s]) - int(t)):
            s -= 1
        ent_bnd.append(s)
    ent_bnd.append(E)

    cores = []
    for c in range(N_CORES):
        e_lo, e_hi = ent_bnd[c], ent_bnd[c + 1]
        segs = np.arange(e_lo, e_hi)
        sizes = (starts[e_lo + 1 : e_hi + 1] - starts[e_lo:e_hi]).astype(np.int64)
        n_edges = int(sizes.sum())
        assert e_hi - e_lo <= E_PAD and n_edges <= NL
        order = np.argsort(-sizes, kind="stable")
        blk_edges = [0] * NBLK
        blk_nseg = [0] * NBLK
        blk_segs = [[] for _ in range(NBLK)]
        for idx in order:
            sz = int(sizes[idx])
            best = -1
            for b in sorted(range(NBLK), key=lambda b: blk_edges[b]):
                if blk_nseg[b] < SPB and blk_edges[b] + sz <= EPB:
                    best = b
                    break
            assert best >= 0, "block packing overflow"
            blk_segs[best].append(int(segs[idx]))
            blk_edges[best] += sz
            blk_nseg[best] += 1
        perm = np.full(NL, -1, dtype=np.int64)
        seg_local = np.full(NL, PAD_SEG, dtype=np.float32)
        row2seg = np.full(E_PAD, -1, dtype=np.int64)
        inv_cnt = np.zeros(E_PAD, dtype=np.float32)
        for b in range(NBLK):
            pos = b * EPB
            for j, s in enumerate(blk_segs[b]):
                row = b * SPB + j
                row2seg[row] = s
                n = int(starts[s + 1] - starts[s])
                if n > 0:
                    inv_cnt[row] = 1.0 / n
                perm[pos : pos + n] = np.arange(starts[s], starts[s + 1])
                seg_local[pos : pos + n] = float(j)
                pos += n
        cores.append(dict(perm=perm, seg_local=seg_local, row2seg=row2seg,
                          inv_cnt=inv_cnt))
    return cores


# --------------------------------------------------------------------------
# device kernel
# --------------------------------------------------------------------------

def _build_nc():
    nc = bass.Bass("TRN2", target_bir_lowering=False, debug=False,
                   num_devices=N_CORES)

    f32, bf, i32 = dt.float32, dt.bfloat16, dt.int32
    din = lambda n, s, d=f32: nc.dram_tensor(n, s, d, kind="ExternalInput")
    mega_d = din("mega", [NL, 3 * D], bf)
    cnt_d = din("cnt", [NL, DC], bf)
    segl_d = din("segl", [NL])
    sc0_d = din("sc0", [NL])
    icnt_d = din("inv_cnt", [E_PAD])
    wsenr_d = din("wsenr", [128, 3 * D], bf)
    # tiled projector weights: [ot, k, 128, 128] (lhsT tiles)
    wtr_d = din("wtr", [NOT, KC_R, 128, 128], bf)
    wte_d = din("wte", [NOT, KC_E, 128, 128], bf)
    orelT_d = nc.dram_tensor("orelT", [OUT, E_PAD], bf, kind="ExternalOutput")
    oentT_d = nc.dram_tensor("oentT", [OUT, E_PAD], bf, kind="ExternalOutput")

    with _TileContextSplitDrain(nc) as tc, ExitStack() as es:
        const = es.enter_context(tc.tile_pool(name="const", bufs=1))
        accp = es.enter_context(tc.tile_pool(name="accp", bufs=1))

        # ---- constants ----
        iota_seg = const.tile([128, 128], bf)
        ident = const.tile([128, 128], bf)
        with tc.tile_pool(name="setup", bufs=1) as setup:
            iota_i = setup.tile([128, 128], i32)
            nc.gpsimd.iota(iota_i[:], pattern=[[1, 128]], base=0,
                           channel_multiplier=0)
            nc.vector.tensor_copy(iota_seg[:], iota_i[:])
            iota_ci = setup.tile([128, 1], i32)
            nc.gpsimd.iota(iota_ci[:], pattern=[[0, 1]], base=0,
                           channel_multiplier=1)
            iota_col = setup.tile([128, 1], f32)
            nc.vector.tensor_copy(iota_col[:], iota_ci[:])
            nc.vector.tensor_scalar(out=ident[:], in0=iota_seg[:],
                                    scalar1=iota_col[:],
                                    scalar2=None, op0=Alu.is_equal)
        wsenr = const.tile([128, 3 * D], bf)
        nc.sync.dma_start(wsenr[:], wsenr_d.ap())
        icnt_sb = const.tile([128, NBLK], f32)
        nc.sync.dma_start(
            icnt_sb[:], icnt_d.ap().rearrange("(b p) -> p b", p=128)
        )
        invd_sb = accp.tile([128, NBLK], f32)

        # resident transposed aggregates: ATr[k] = [128 (k-dim), 1280 (seg)]
        ATr = [accp.tile([128, E_PAD], bf, name=f"ATr{k}", tag=f"ATr{k}")
               for k in range(KC_R)]
        ATe = [accp.tile([128, E_PAD], bf, name=f"ATe{k}", tag=f"ATe{k}")
               for k in range(KC_E)]

        HD = (CH // 2) * 3 * D  # half-block mega width (5 chunks x 2304)
        HDC = (CH // 2) * DC

        # ================= Phase A + staircase =================
        SC_OTS = list(range(18))  # ots whose bg0 cells run during phase A
        parts = None  # filled below

        with tc.tile_pool(name="edges", bufs=3) as edges, \
             tc.tile_pool(name="chunkp", bufs=4) as chunkp, \
             tc.tile_pool(name="ohp", bufs=12) as ohp, \
             tc.tile_pool(name="scrp", bufs=2) as scrp, \
             tc.tile_pool(name="evac", bufs=2) as evac, \
             tc.tile_pool(name="wpool", bufs=2) as wpool, \
             tc.tile_pool(name="outp", bufs=4) as outp, \
             tc.tile_pool(name="psagg", bufs=1, space="PSUM") as psagg, \
             tc.tile_pool(name="pp", bufs=2, space="PSUM") as pp, \
             tc.tile_pool(name="cellps", bufs=2, space="PSUM") as cellps:

            parts = (
                ("r", wtr_d, ATr, KC_R, orelT_d),
                ("e", wte_d, ATe, KC_E, oentT_d),
            )
            BGS = ((0, 512), (512, 1024), (1024, 1280))

            def emit_cell(pi, ot, bg, wt):
                tag, wt_d, ATl, KC, o_d = parts[pi]
                lo, hi = BGS[bg]
                w = hi - lo
                ps = cellps.tile([128, 512], f32, tag="cell")
                for k in range(KC):
                    nc.tensor.matmul(ps[:, 0:w], wt[:, k * 128 : (k + 1) * 128],
                                     ATl[k][:, lo:hi],
                                     start=(k == 0), stop=(k == KC - 1))
                stage = outp.tile([128, 512], bf, tag="cst")
                if (ot + bg) % 2 == 0:
                    nc.vector.tensor_copy(stage[:, 0:w], ps[:, 0:w])
                else:
                    nc.scalar.activation(stage[:, 0:w], ps[:, 0:w], Act.Copy)
                nc.sync.dma_start(
                    o_d.ap()[ot * 128 : (ot + 1) * 128, lo:hi], stage[:, 0:w]
                )

            def load_wt(pi, ot):
                tag, wt_d, ATl, KC, o_d = parts[pi]
                wt = wpool.tile([128, KC * 128], bf, tag=f"wt{tag}")
                nc.gpsimd.dma_start(
                    wt[:], wt_d.ap()[ot].rearrange("k p c -> p k c")
                )
                return wt

            for b in range(NBLK):
                halves = []
                for hb in range(2):
                    r0 = b * EPB + hb * (EPB // 2)
                    r1 = r0 + EPB // 2
                    megah = edges.tile([128, HD], bf, tag="megah")
                    nc.gpsimd.dma_start(
                        megah[:],
                        mega_d.ap()[r0:r1, :].rearrange("(p j) d -> p j d", j=CH // 2),
                    )
                    cnth = edges.tile([128, HDC], bf, tag="cnth")
                    nc.gpsimd.dma_start(
                        cnth[:],
                        cnt_d.ap()[r0:r1, :].rearrange("(p j) d -> p j d", j=CH // 2),
                    )
                    slh = edges.tile([128, CH // 2], f32, tag="slh")
                    nc.sync.dma_start(
                        slh[:], segl_d.ap()[r0:r1].rearrange("(p j) -> p j", j=CH // 2))
                    sch = edges.tile([128, CH // 2], f32, tag="sch")
                    nc.sync.dma_start(
                        sch[:], sc0_d.ap()[r0:r1].rearrange("(p j) -> p j", j=CH // 2))
                    halves.append((megah, cnth, slh, sch))

                # ---- pass 1: rel|cnt aggregation (+denominator) ----
                ps_rc = psagg.tile([128, 2048], f32, tag="ps")
                ohs = []
                for j in range(CH):
                    megah, cnth, slh, sch = halves[j // 5]
                    jj = j % 5
                    mj = megah[:, jj * 3 * D : (jj + 1) * 3 * D]
                    rj = megah[:, jj * 3 * D + 2 * D : (jj + 1) * 3 * D]
                    cj = cnth[:, jj * DC : jj * DC + DC]
                    scr = scrp.tile([128, 3 * D], bf, tag="scr")
                    sa = chunkp.tile([128, 1], f32, tag="sa")
                    nc.vector.scalar_tensor_tensor(
                        out=scr[:], in0=mj, scalar=1.0, in1=wsenr[:],
                        op0=Alu.mult, op1=Alu.mult, accum_out=sa[:])
                    ex_ = chunkp.tile([128, 1], f32, tag="ex_")
                    nc.scalar.activation(ex_[:], sa[:], Act.Exp,
                                         bias=sch[:, jj : jj + 1])
                    oh = ohp.tile([128, 128], bf, tag="oh")
                    nc.vector.tensor_scalar(out=oh[:], in0=iota_seg[:],
                                            scalar1=slh[:, jj : jj + 1],
                                            scalar2=None, op0=Alu.is_equal)
                    ohs.append(oh)
                    ohx = chunkp.tile([128, 128], bf, tag="ohx")
                    nc.vector.tensor_scalar(out=ohx[:], in0=iota_seg[:],
                                            scalar1=slh[:, jj : jj + 1],
                                            scalar2=ex_[:],
                                            op0=Alu.is_equal, op1=Alu.mult)
                    st, sp = (j == 0), (j == CH - 1)
                    nc.tensor.matmul(ps_rc[:, 0:512], ohx[:], rj[:, 0:512],
                                     start=st, stop=sp)
                    nc.tensor.matmul(ps_rc[:, 512:768], ohx[:], rj[:, 512:768],
                                     start=st, stop=sp)
                    nc.tensor.matmul(ps_rc[:, 1024:1536], ohx[:], cj[:, 0:512],
                                     start=st, stop=sp)
                    nc.tensor.matmul(ps_rc[:, 1536:1856], ohx[:], cj[:, 512:832],
                                     start=st, stop=sp)

                # epilogue 1: invd + normalized rel|cnt evac + transposes
                dmx = chunkp.tile([128, 1], f32, tag="dmx")
                nc.vector.tensor_scalar(out=dmx[:], in0=ps_rc[:, 1792:1793],
                                        scalar1=1e-30, scalar2=None, op0=Alu.max)
                nc.vector.reciprocal(invd_sb[:, b : b + 1], dmx[:])
                rcsb = evac.tile([128, 2 * D], bf, tag="rcsb")
                nc.scalar.activation(rcsb[:, 0:768], ps_rc[:, 0:768], Act.Copy,
                                     scale=invd_sb[:, b : b + 1])
                nc.scalar.activation(rcsb[:, 768:1536], ps_rc[:, 1024:1792],
                                     Act.Copy, scale=invd_sb[:, b : b + 1])

                bs = slice(b * 128, (b + 1) * 128)
                for k in range(KC_R):
                    pt = pp.tile([128, 512], bf, tag="pp")
                    nc.tensor.transpose(pt[:, 0:128],
                                        rcsb[:, k * 128 : (k + 1) * 128],
                                        ident[:])
                    if k % 2 == 0:
                        nc.vector.tensor_copy(ATr[k][:, bs], pt[:, 0:128])
                    else:
                        nc.scalar.activation(ATr[k][:, bs], pt[:, 0:128],
                                             Act.Copy)

                # ---- pass 2: ent aggregation (psum banks reused) ----
                ps_ed = psagg.tile([128, 2048], f32, tag="ps")
                for j in range(CH):
                    megah, cnth, slh, sch = halves[j // 5]
                    jj = j % 5
                    ej = megah[:, jj * 3 * D : jj * 3 * D + D]
                    st, sp = (j == 0), (j == CH - 1)
                    nc.tensor.matmul(ps_ed[:, 0:512], ohs[j][:], ej[:, 0:512],
                                     start=st, stop=sp)
                    nc.tensor.matmul(ps_ed[:, 512:768], ohs[j][:], ej[:, 512:768],
                                     start=st, stop=sp)
                ohs = None
                edsb = evac.tile([128, D], bf, tag="edsb")
                nc.scalar.activation(edsb[:], ps_ed[:, 0:768], Act.Copy,
                                     scale=icnt_sb[:, b : b + 1])
                for k in range(KC_E):
                    pt = pp.tile([128, 512], bf, tag="pp")
                    nc.tensor.transpose(pt[:, 0:128],
                                        edsb[:, k * 128 : (k + 1) * 128],
                                        ident[:])
                    if k % 2 == 0:
                        nc.vector.tensor_copy(ATe[k][:, bs], pt[:, 0:128])
                    else:
                        nc.scalar.activation(ATe[k][:, bs], pt[:, 0:128],
                                             Act.Copy)

                # ---- staircase: bg0 cells for 3 ots per gap after block 3 ----
                if 4 <= b <= 9:
                    for ot in SC_OTS[3 * (b - 4) : 3 * (b - 4) + 3]:
                        wtr_t = load_wt(0, ot)
                        wte_t = load_wt(1, ot)
                        emit_cell(0, ot, 0, wtr_t)
                        emit_cell(1, ot, 0, wte_t)

            # ---- tail: remaining cells ----
            for ot in range(NOT):
                wtr_t = load_wt(0, ot)
                wte_t = load_wt(1, ot)
                bgs = (1, 2) if ot in SC_OTS else (0, 1, 2)
                for bg in bgs:
                    emit_cell(0, ot, bg, wtr_t)
                for bg in bgs:
                    emit_cell(1, ot, bg, wte_t)
    return nc


_NC_CACHE = None


def _get_nc():
    global _NC_CACHE
    if _NC_CACHE is None:
        _NC_CACHE = _build_nc()
    return _NC_CACHE


# --------------------------------------------------------------------------
# entry point
# --------------------------------------------------------------------------

def kernel(prompt_embs, entity_embs, neighbor_embs, relation_embs,
           count_table, scorer_W, scorer_b, rel_W, rel_b, ent_W, ent_b,
           counts, prompt_indices, entity_indices):
    from concourse.bass_utils import run_bass_kernel_spmd

    prompt_embs = np.asarray(prompt_embs, dtype=np.float32)
    entity_embs = np.asarray(entity_embs, dtype=np.float32)
    neighbor_embs = np.asarray(neighbor_embs, dtype=np.float32)
    relation_embs = np.asarray(relation_embs, dtype=np.float32)
    count_table = np.asarray(count_table, dtype=np.float32)
    scorer_W = np.asarray(scorer_W, dtype=np.float32)
    scorer_b = np.asarray(scorer_b, dtype=np.float32)
    rel_W = np.asarray(rel_W, dtype=np.float32)
    rel_b = np.asarray(rel_b, dtype=np.float32)
    ent_W = np.asarray(ent_W, dtype=np.float32)
    ent_b = np.asarray(ent_b, dtype=np.float32)
    counts = np.asarray(counts)
    prompt_indices = np.asarray(prompt_indices)
    entity_indices = np.asarray(entity_indices)

    cores = _shard_and_pack(entity_indices)

    # replicated (weight-derived) host prep
    w = scorer_W[0]
    w1, w2, w3, w4, w5 = (w[i * D : (i + 1) * D] for i in range(5))
    pscore = (prompt_embs * w1[None, :]).sum(1) + scorer_b[0]     # fold bias
    cscore = (count_table * w5[None, :]).sum(1)
    wsenr = np.broadcast_to(
        np.concatenate([w2, w3, w4]).astype(BF16), (128, 3 * D)).copy()
    # tiled lhsT weight tiles: wtr[ot, k, kl, ol] = rel_W[ot*128+ol, k*128+kl]
    wtr = np.ascontiguousarray(
        rel_W.T.reshape(KC_R, 128, NOT, 128).transpose(2, 0, 1, 3)
    ).astype(BF16)
    wte = np.ascontiguousarray(
        ent_W.T.reshape(KC_E, 128, NOT, 128).transpose(2, 0, 1, 3)
    ).astype(BF16)

    mega16 = np.empty((N, 3 * D), dtype=BF16)
    mega16[:, 0:D] = entity_embs.astype(BF16)
    mega16[:, D : 2 * D] = neighbor_embs.astype(BF16)
    mega16[:, 2 * D :] = relation_embs.astype(BF16)
    cnt16 = np.zeros((N, DC), dtype=BF16)
    cnt16[:, 0:D] = count_table.astype(BF16)[counts]  # [N, D] gather
    cnt16[:, D] = BF16(1.0)                           # denominator ones col
    sc0_full = (pscore[prompt_indices] + cscore[counts]).astype(np.float32)

    in_maps = []
    for core in cores:
        perm = core["perm"]
        valid = perm >= 0
        src = np.where(valid, perm, 0)

        def take2d(a16):
            out = a16[src]
            out[~valid] = 0.0
            return np.ascontiguousarray(out)

        sc0 = sc0_full[src]
        sc0[~valid] = 0.0

        in_maps.append(dict(
            mega=take2d(mega16), cnt=take2d(cnt16),
            segl=core["seg_local"], sc0=np.ascontiguousarray(sc0),
            inv_cnt=core["inv_cnt"],
            wsenr=wsenr, wtr=wtr, wte=wte,
        ))

    nc = _get_nc()
    res = run_bass_kernel_spmd(nc, in_maps, list(range(N_CORES)))

    rel_out = np.zeros((E, OUT), np.float32)
    ent_out = np.zeros((E, OUT), np.float32)
    for c, core in enumerate(cores):
        rows = core["row2seg"]
        mask = rows >= 0
        rel_out[rows[mask]] = res.results[c]["orelT"].T[mask].astype(np.float32)
        ent_out[rows[mask]] = res.results[c]["oentT"].T[mask].astype(np.float32)
    rel_out += rel_b[None, :]
    ent_out += ent_b[None, :]
    return rel_out, ent_out


# revision 13
# speedup vs baseline: 1.4931x; 1.0118x over previous
"""EntityEncoder (gnn_message_passing) Trainium2 kernel — 8-core SPMD, v2.

Strategy: edges pre-partitioned on host into 8 contiguous entity-aligned
shards (entity_indices sorted => no cross-core collectives). Per core,
segments LPT-packed into 10 blocks of <=128 segments / <=1280 edges.

v2 changes vs v1:
  - all embedding streams converted to bf16 on host (halves HBM reads)
  - count embeddings gathered on host into a 4th edge stream (removes
    one-hot count/prompt vector work and the count-table matmul chain)
  - prompt/count scorer contributions folded on host into one per-edge
    scalar (exp bias)
  - two-phase device schedule: aggregation (one-hot matmuls + PE
    transposes into resident transposed aggregates), then projection
    with weight-stationary matmuls (one LDW per 1280 streamed cols)
  - outputs written transposed [OUT, E_PAD] in bf16; bias + transpose
    + scatter on host
"""
import sys
import numpy as np
import ml_dtypes

for _p in ("/root/.axon_site", "/root/.axon_site/_ro/trn_rl_repo",
           "/root/.axon_site/_ro/pypackages"):
    if _p not in sys.path:
        sys.path.append(_p)

import bass_rust
import concourse.bass as bass
import concourse.mybir as mybir
import concourse.tile as tile
from concourse.vector_clock import ScopedClock
from contextlib import ExitStack

BF16 = ml_dtypes.bfloat16
dt = mybir.dt
Alu = mybir.AluOpType
Act = mybir.ActivationFunctionType

# problem shape (hardcoded per contest contract)
N_CORES = 8
N = 100_000
P = 64
E = 10_000
D = 768
C = 1000
OUT = 5120
# per-core packing
NBLK = 10
SPB = 128                # segs per block
CH = 10                  # chunks (of 128 edges) per block
EPB = CH * 128           # edges per block = 1280
NL = NBLK * EPB          # 12800 edge slots per core
E_PAD = NBLK * SPB       # 1280 seg slots per core
KC_R = 12                # rel|cnt contraction chunks (1536/128)
KC_E = 6                 # ent contraction chunks (768/128)
DC = 832                 # cnt stream width: 768 emb + ones col at 768
NOT = OUT // 128         # 40 output tiles of 128 cols
PAD_SEG = 999.0


class _TileContextSplitDrain(tile.TileContext):
    """This container's walrus accepts only ONE sync wait per instruction
    ("Too many sync wait commands" in setupSyncWait). Split every extra wait
    onto a standalone same-engine NoOp placed immediately before the
    instruction — identical semantics, one wait per instruction."""

    def _lower_ordered_insts(self, ordered):
        for insts in ordered.values():
            if not any(
                i.sync_info is not None and len(i.sync_info.on_wait) > 1
                for i in insts
            ):
                continue
            new = []
            for inst in insts:
                si = inst.sync_info
                if si is not None and len(si.on_wait) > 1:
                    waits = list(si.on_wait)
                    for w in waits[:-1]:
                        nop = bass_rust.InstNoOp(
                            name=self.nc.get_next_instruction_name(),
                            ins=[], outs=[])
                        nop.engine = inst.engine
                        nop.sync_info = bass_rust.SyncInfo(
                            on_wait=[w], on_update=[])
                        new.append(nop)
                    si.on_wait = waits[-1:]
                new.append(inst)
            insts[:] = new
        return super()._lower_ordered_insts(ordered)

    def _drain_and_barrier(self, tick_clock, wait_clock):
        nc = self.nc
        drain_inst = nc.sync.drain()
        wait_clock.add_sem_waits(
            drain_inst.ins, ScopedClock({None: tick_clock.global_clock})
        )
        si = drain_inst.ins.sync_info
        if si is not None and len(si.on_wait) > 1:
            waits = list(si.on_wait)
            si.on_wait = waits[:1]
            for w in waits[1:]:
                n = nc.sync.nop()
                n.ins.sync_info = bass_rust.SyncInfo(on_wait=[w], on_update=[])
        nc.all_engine_barrier()
        assert self.sems is not None
        popped = nc._tile_sem_poison_stack.pop()
        assert popped is self._sem_poison
        nc.clear_and_free_semaphores(list(self.sems.allocated().values()))
        nc.all_engine_barrier()


# --------------------------------------------------------------------------
# host-side sharding / packing
# --------------------------------------------------------------------------

def _shard_and_pack(entity_indices):
    Nn = entity_indices.shape[0]
    starts = np.searchsorted(entity_indices, np.arange(E + 1))
    ideal = (np.arange(1, N_CORES) * Nn) // N_CORES
    ent_bnd = [0]
    for t in ideal:
        s = int(np.searchsorted(starts, t))
        if s > 0 and abs(int(starts[s - 1]) - int(t)) < abs(int(starts[s]) - int(t)):
            s -= 1
        ent_bnd.append(s)
    ent_bnd.append(E)

    cores = []
    for c in range(N_CORES):
        e_lo, e_hi = ent_bnd[c], ent_bnd[c + 1]
        segs = np.arange(e_lo, e_hi)
        sizes = (starts[e_lo + 1 : e_hi + 1] - starts[e_lo:e_hi]).astype(np.int64)
        n_edges = int(sizes.sum())
        assert e_hi - e_lo <= E_PAD and n_edges <= NL
        order = np.argsort(-sizes, kind="stable")
        blk_edges = [0] * NBLK
        blk_nseg = [0] * NBLK
        blk_segs = [[] for _ in range(NBLK)]
        for idx in order:
            sz = int(sizes[idx])
            best = -1
            for b in sorted(range(NBLK), key=lambda b: blk_edges[b]):
                if blk_nseg[b] < SPB and blk_edges[b] + sz <= EPB:
                    best = b
                    break
            assert best >= 0, "block packing overflow"
            blk_segs[best].append(int(segs[idx]))
            blk_edges[best] += sz
            blk_nseg[best] += 1
        perm = np.full(NL, -1, dtype=np.int64)
        seg_local = np.full(NL, PAD_SEG, dtype=np.float32)
        row2seg = np.full(E_PAD, -1, dtype=np.int64)
        inv_cnt = np.zeros(E_PAD, dtype=np.float32)
        for b in range(NBLK):
            pos = b * EPB
            for j, s in enumerate(blk_segs[b]):
                row = b * SPB + j
                row2seg[row] = s
                n = int(starts[s + 1] - starts[s])
                if n > 0:
                    inv_cnt[row] = 1.0 / n
                perm[pos : pos + n] = np.arange(starts[s], starts[s + 1])
                seg_local[pos : pos + n] = float(j)
                pos += n
        cores.append(dict(perm=perm, seg_local=seg_local, row2seg=row2seg,
                          inv_cnt=inv_cnt))
    return cores


# --------------------------------------------------------------------------
# device kernel
# --------------------------------------------------------------------------

def _build_nc():
    nc = bass.Bass("TRN2", target_bir_lowering=False, debug=False,
                   num_devices=N_CORES)

    f32, bf, i32 = dt.float32, dt.bfloat16, dt.int32
    din = lambda n, s, d=f32: nc.dram_tensor(n, s, d, kind="ExternalInput")
    mega_d = din("mega", [2 * NBLK, 128, (CH // 2) * 3 * D], bf)
    cnt_d = din("cnt", [2 * NBLK, 128, (CH // 2) * DC], bf)
    segl_d = din("segl", [NL])
    sc0_d = din("sc0", [NL])
    icnt_d = din("inv_cnt", [E_PAD])
    wsenr_d = din("wsenr", [128, 3 * D], bf)
    # tiled projector weights: [ot, k, 128, 128] (lhsT tiles)
    wtr_d = din("wtr", [NOT, KC_R, 128, 128], bf)
    wte_d = din("wte", [NOT, KC_E, 128, 128], bf)
    orelT_d = nc.dram_tensor("orelT", [OUT, E_PAD], bf, kind="ExternalOutput")
    oentT_d = nc.dram_tensor("oentT", [OUT, E_PAD], bf, kind="ExternalOutput")

    with _TileContextSplitDrain(nc) as tc, ExitStack() as es:
        const = es.enter_context(tc.tile_pool(name="const", bufs=1))
        accp = es.enter_context(tc.tile_pool(name="accp", bufs=1))

        # ---- constants ----
        iota_seg = const.tile([128, 128], bf)
        ident = const.tile([128, 128], bf)
        with tc.tile_pool(name="setup", bufs=1) as setup:
            iota_i = setup.tile([128, 128], i32)
            nc.gpsimd.iota(iota_i[:], pattern=[[1, 128]], base=0,
                           channel_multiplier=0)
            nc.vector.tensor_copy(iota_seg[:], iota_i[:])
            iota_ci = setup.tile([128, 1], i32)
            nc.gpsimd.iota(iota_ci[:], pattern=[[0, 1]], base=0,
                           channel_multiplier=1)
            iota_col = setup.tile([128, 1], f32)
            nc.vector.tensor_copy(iota_col[:], iota_ci[:])
            nc.vector.tensor_scalar(out=ident[:], in0=iota_seg[:],
                                    scalar1=iota_col[:],
                                    scalar2=None, op0=Alu.is_equal)
        wsenr = const.tile([128, 3 * D], bf)
        nc.sync.dma_start(wsenr[:], wsenr_d.ap())
        icnt_sb = const.tile([128, NBLK], f32)
        nc.sync.dma_start(
            icnt_sb[:], icnt_d.ap().rearrange("(b p) -> p b", p=128)
        )
        invd_sb = accp.tile([128, NBLK], f32)

        # resident transposed aggregates: ATr[k] = [128 (k-dim), 1280 (seg)]
        ATr = [accp.tile([128, E_PAD], bf, name=f"ATr{k}", tag=f"ATr{k}")
               for k in range(KC_R)]
        ATe = [accp.tile([128, E_PAD], bf, name=f"ATe{k}", tag=f"ATe{k}")
               for k in range(KC_E)]

        HD = (CH // 2) * 3 * D  # half-block mega width (5 chunks x 2304)
        HDC = (CH // 2) * DC

        # ================= Phase A + staircase =================
        SC_OTS = list(range(NOT))  # all bg0 cells run during phase A
        GAP_AT = [0, 7, 14, 21, 28, 34, 40]  # SC_OTS slices per gap
        parts = None  # filled below

        with tc.tile_pool(name="edges", bufs=3) as edges, \
             tc.tile_pool(name="chunkp", bufs=4) as chunkp, \
             tc.tile_pool(name="ohp", bufs=12) as ohp, \
             tc.tile_pool(name="scrp", bufs=2) as scrp, \
             tc.tile_pool(name="evac", bufs=2) as evac, \
             tc.tile_pool(name="wpool", bufs=2) as wpool, \
             tc.tile_pool(name="outp", bufs=4) as outp, \
             tc.tile_pool(name="psagg", bufs=1, space="PSUM") as psagg, \
             tc.tile_pool(name="pp", bufs=2, space="PSUM") as pp, \
             tc.tile_pool(name="cellps", bufs=2, space="PSUM") as cellps:

            parts = (
                ("r", wtr_d, ATr, KC_R, orelT_d),
                ("e", wte_d, ATe, KC_E, oentT_d),
            )
            BGS = ((0, 512), (512, 1024), (1024, 1280))

            def emit_cell(pi, ot, bg, wt):
                tag, wt_d, ATl, KC, o_d = parts[pi]
                lo, hi = BGS[bg]
                w = hi - lo
                ps = cellps.tile([128, 512], f32, tag="cell")
                for k in range(KC):
                    nc.tensor.matmul(ps[:, 0:w], wt[:, k * 128 : (k + 1) * 128],
                                     ATl[k][:, lo:hi],
                                     start=(k == 0), stop=(k == KC - 1))
                stage = outp.tile([128, 512], bf, tag="cst")
                if (ot + bg) % 2 == 0:
                    nc.vector.tensor_copy(stage[:, 0:w], ps[:, 0:w])
                else:
                    nc.scalar.activation(stage[:, 0:w], ps[:, 0:w], Act.Copy)
                nc.sync.dma_start(
                    o_d.ap()[ot * 128 : (ot + 1) * 128, lo:hi], stage[:, 0:w]
                )

            def load_wt(pi, ot):
                tag, wt_d, ATl, KC, o_d = parts[pi]
                wt = wpool.tile([128, KC * 128], bf, tag=f"wt{tag}")
                nc.gpsimd.dma_start(
                    wt[:], wt_d.ap()[ot].rearrange("k p c -> p k c")
                )
                return wt

            for b in range(NBLK):
                halves = []
                for hb in range(2):
                    r0 = b * EPB + hb * (EPB // 2)
                    r1 = r0 + EPB // 2
                    megah = edges.tile([128, HD], bf, tag="megah")
                    nc.sync.dma_start(megah[:], mega_d.ap()[2 * b + hb])
                    cnth = edges.tile([128, HDC], bf, tag="cnth")
                    nc.scalar.dma_start(cnth[:], cnt_d.ap()[2 * b + hb])
                    slh = edges.tile([128, CH // 2], f32, tag="slh")
                    nc.sync.dma_start(
                        slh[:], segl_d.ap()[r0:r1].rearrange("(p j) -> p j", j=CH // 2))
                    sch = edges.tile([128, CH // 2], f32, tag="sch")
                    nc.sync.dma_start(
                        sch[:], sc0_d.ap()[r0:r1].rearrange("(p j) -> p j", j=CH // 2))
                    halves.append((megah, cnth, slh, sch))

                # ---- pass 1: rel|cnt aggregation (+denominator) ----
                ps_rc = psagg.tile([128, 2048], f32, tag="ps")
                ohs = []
                for j in range(CH):
                    megah, cnth, slh, sch = halves[j // 5]
                    jj = j % 5
                    mj = megah[:, jj * 3 * D : (jj + 1) * 3 * D]
                    rj = megah[:, jj * 3 * D + 2 * D : (jj + 1) * 3 * D]
                    cj = cnth[:, jj * DC : jj * DC + DC]
                    scr = scrp.tile([128, 3 * D], bf, tag="scr")
                    sa = chunkp.tile([128, 1], f32, tag="sa")
                    nc.vector.scalar_tensor_tensor(
                        out=scr[:], in0=mj, scalar=1.0, in1=wsenr[:],
                        op0=Alu.mult, op1=Alu.mult, accum_out=sa[:])
                    ex_ = chunkp.tile([128, 1], f32, tag="ex_")
                    nc.scalar.activation(ex_[:], sa[:], Act.Exp,
                                         bias=sch[:, jj : jj + 1])
                    oh = ohp.tile([128, 128], bf, tag="oh")
                    nc.vector.tensor_scalar(out=oh[:], in0=iota_seg[:],
                                            scalar1=slh[:, jj : jj + 1],
                                            scalar2=None, op0=Alu.is_equal)
                    ohs.append(oh)
                    ohx = chunkp.tile([128, 128], bf, tag="ohx")
                    nc.vector.tensor_scalar(out=ohx[:], in0=iota_seg[:],
                                            scalar1=slh[:, jj : jj + 1],
                                            scalar2=ex_[:],
                                            op0=Alu.is_equal, op1=Alu.mult)
                    st, sp = (j == 0), (j == CH - 1)
                    nc.tensor.matmul(ps_rc[:, 0:512], ohx[:], rj[:, 0:512],
                                     start=st, stop=sp)
                    nc.tensor.matmul(ps_rc[:, 512:768], ohx[:], rj[:, 512:768],
                                     start=st, stop=sp)
                    nc.tensor.matmul(ps_rc[:, 1024:1536], ohx[:], cj[:, 0:512],
                                     start=st, stop=sp)
                    nc.tensor.matmul(ps_rc[:, 1536:1856], ohx[:], cj[:, 512:832],
                                     start=st, stop=sp)

                # epilogue 1: invd + normalized rel|cnt evac + transposes
                dmx = chunkp.tile([128, 1], f32, tag="dmx")
                nc.vector.tensor_scalar(out=dmx[:], in0=ps_rc[:, 1792:1793],
                                        scalar1=1e-30, scalar2=None, op0=Alu.max)
                nc.vector.reciprocal(invd_sb[:, b : b + 1], dmx[:])
                rcsb = evac.tile([128, 2 * D], bf, tag="rcsb")
                nc.scalar.activation(rcsb[:, 0:768], ps_rc[:, 0:768], Act.Copy,
                                     scale=invd_sb[:, b : b + 1])
                nc.scalar.activation(rcsb[:, 768:1536], ps_rc[:, 1024:1792],
                                     Act.Copy, scale=invd_sb[:, b : b + 1])

                bs = slice(b * 128, (b + 1) * 128)
                for k in range(KC_R):
                    pt = pp.tile([128, 512], bf, tag="pp")
                    nc.tensor.transpose(pt[:, 0:128],
                                        rcsb[:, k * 128 : (k + 1) * 128],
                                        ident[:])
                    nc.scalar.activation(ATr[k][:, bs], pt[:, 0:128],
                                         Act.Copy)

                # ---- pass 2: ent aggregation (psum banks reused) ----
                ps_ed = psagg.tile([128, 2048], f32, tag="ps")
                for j in range(CH):
                    megah, cnth, slh, sch = halves[j // 5]
                    jj = j % 5
                    ej = megah[:, jj * 3 * D : jj * 3 * D + D]
                    st, sp = (j == 0), (j == CH - 1)
                    nc.tensor.matmul(ps_ed[:, 0:512], ohs[j][:], ej[:, 0:512],
                                     start=st, stop=sp)
                    nc.tensor.matmul(ps_ed[:, 512:768], ohs[j][:], ej[:, 512:768],
                                     start=st, stop=sp)
                ohs = None
                edsb = evac.tile([128, D], bf, tag="edsb")
                nc.scalar.activation(edsb[:], ps_ed[:, 0:768], Act.Copy,
                                     scale=icnt_sb[:, b : b + 1])
                for k in range(KC_E):
                    pt = pp.tile([128, 512], bf, tag="pp")
                    nc.tensor.transpose(pt[:, 0:128],
                                        edsb[:, k * 128 : (k + 1) * 128],
                                        ident[:])
                    nc.scalar.activation(ATe[k][:, bs], pt[:, 0:128],
                                         Act.Copy)

                # ---- staircase: bg0 cells for 3 ots per gap after block 3 ----
                if 4 <= b <= 9:
                    for ot in SC_OTS[GAP_AT[b - 4] : GAP_AT[b - 3]]:
                        wtr_t = load_wt(0, ot)
                        wte_t = load_wt(1, ot)
                        emit_cell(0, ot, 0, wtr_t)
                        emit_cell(1, ot, 0, wte_t)

            # ---- tail: remaining cells ----
            for ot in range(NOT):
                wtr_t = load_wt(0, ot)
                wte_t = load_wt(1, ot)
                bgs = (1, 2) if ot in SC_OTS else (0, 1, 2)
                for bg in bgs:
                    emit_cell(0, ot, bg, wtr_t)
                for bg in bgs:
                    emit_cell(1, ot, bg, wte_t)
    return nc


_NC_CACHE = None


def _get_nc():
    global _NC_CACHE
    if _NC_CACHE is None:
        _NC_CACHE = _build_nc()
    return _NC_CACHE


# --------------------------------------------------------------------------
# entry point
# --------------------------------------------------------------------------

def kernel(prompt_embs, entity_embs, neighbor_embs, relation_embs,
           count_table, scorer_W, scorer_b, rel_W, rel_b, ent_W, ent_b,
           counts, prompt_indices, entity_indices):
    from concourse.bass_utils import run_bass_kernel_spmd

    prompt_embs = np.asarray(prompt_embs, dtype=np.float32)
    entity_embs = np.asarray(entity_embs, dtype=np.float32)
    neighbor_embs = np.asarray(neighbor_embs, dtype=np.float32)
    relation_embs = np.asarray(relation_embs, dtype=np.float32)
    count_table = np.asarray(count_table, dtype=np.float32)
    scorer_W = np.asarray(scorer_W, dtype=np.float32)
    scorer_b = np.asarray(scorer_b, dtype=np.float32)
    rel_W = np.asarray(rel_W, dtype=np.float32)
    rel_b = np.asarray(rel_b, dtype=np.float32)
    ent_W = np.asarray(ent_W, dtype=np.float32)
    ent_b = np.asarray(ent_b, dtype=np.float32)
    counts = np.asarray(counts)
    prompt_indices = np.asarray(prompt_indices)
    entity_indices = np.asarray(entity_indices)

    cores = _shard_and_pack(entity_indices)

    # replicated (weight-derived) host prep
    w = scorer_W[0]
    w1, w2, w3, w4, w5 = (w[i * D : (i + 1) * D] for i in range(5))
    pscore = (prompt_embs * w1[None, :]).sum(1) + scorer_b[0]     # fold bias
    cscore = (count_table * w5[None, :]).sum(1)
    wsenr = np.broadcast_to(
        np.concatenate([w2, w3, w4]).astype(BF16), (128, 3 * D)).copy()
    # tiled lhsT weight tiles: wtr[ot, k, kl, ol] = rel_W[ot*128+ol, k*128+kl]
    wtr = np.ascontiguousarray(
        rel_W.T.reshape(KC_R, 128, NOT, 128).transpose(2, 0, 1, 3)
    ).astype(BF16)
    wte = np.ascontiguousarray(
        ent_W.T.reshape(KC_E, 128, NOT, 128).transpose(2, 0, 1, 3)
    ).astype(BF16)

    mega16 = np.empty((N, 3 * D), dtype=BF16)
    mega16[:, 0:D] = entity_embs.astype(BF16)
    mega16[:, D : 2 * D] = neighbor_embs.astype(BF16)
    mega16[:, 2 * D :] = relation_embs.astype(BF16)
    cnt16 = np.zeros((N, DC), dtype=BF16)
    cnt16[:, 0:D] = count_table.astype(BF16)[counts]  # [N, D] gather
    cnt16[:, D] = BF16(1.0)                           # denominator ones col
    sc0_full = (pscore[prompt_indices] + cscore[counts]).astype(np.float32)

    in_maps = []
    for core in cores:
        perm = core["perm"]
        valid = perm >= 0
        src = np.where(valid, perm, 0)

        def take2d(a16):
            out = a16[src]
            out[~valid] = 0.0
            return np.ascontiguousarray(out)

        sc0 = sc0_full[src]
        sc0[~valid] = 0.0

        in_maps.append(dict(
            mega=take2d(mega16).reshape(2 * NBLK, 128, (CH // 2) * 3 * D),
            cnt=take2d(cnt16).reshape(2 * NBLK, 128, (CH // 2) * DC),
            segl=core["seg_local"], sc0=np.ascontiguousarray(sc0),
            inv_cnt=core["inv_cnt"],
            wsenr=wsenr, wtr=wtr, wte=wte,
        ))

    nc = _get_nc()
    res = run_bass_kernel_spmd(nc, in_maps, list(range(N_CORES)))

    rel_out = np.zeros((E, OUT), np.float32)
    ent_out = np.zeros((E, OUT), np.float32)
    for c, core in enumerate(cores):
        rows = core["row2seg"]
        mask = rows >= 0
        rel_out[rows[mask]] = res.results[c]["orelT"].T[mask].astype(np.float32)
        ent_out[rows[mask]] = res.results[c]["oentT"].T[mask].astype(np.float32)
    rel_out += rel_b[None, :]
    ent_out += ent_b[None, :]
    return rel_out, ent_out
